# revision 9
# baseline (speedup 1.0000x reference)
"""Trainium2 Bass kernel for nn_DNM_Linear_M3 (dendritic-neuron MLP).

Reference computation (B=64, OUT=512, M=5, IN=1024):
    s = sigmoid(0.5*(x[b,i]*W[o,m,i] - q))      # q constant
    d[b,o,m] = sum_i s[b,o,m,i] * W2[i]
    y[b,o]   = sum_m sigmoid(d[b,o,m])
    out      = k*(y - qs)

Sharding: tensor-parallel over OUT across 8 cores (64 out-values/core).

Per-core dataflow (partition dim = input-dim chunk of 128, IC=8 chunks):
  VectorE  t[i, (b,om)] = W^T[i,om] * x^T[i,b]   bf16 tensor_scalar (4x mode)
  ScalarE  s = sigmoid(0.5*t - 0.5*q)            fused scale/bias, big tiles
  TensorE  d[(b,om)] += W2_chunk^T @ s           PSUM-accumulated over chunks
  DMA      reshape d -> [b, om] partitions
  ScalarE/VectorE  sigmoid(d), sum over m, k*(y-qs)
"""

import numpy as np
from contextlib import ExitStack
from ml_dtypes import bfloat16

import concourse.bass as bass
import concourse.tile as tile
from concourse import bacc, mybir
from concourse import bass_utils

# Problem shape (hardcoded per task contract)
B, OUT, M, IN = 64, 512, 5, 1024
NCORES = 8
OL = OUT // NCORES          # 64 out-values per core
OML = OL * M                # 320 (o,m) pairs per core
P = 128                     # partitions
IC = IN // P                # 8 input chunks
BB = 8                      # batch values per stripe
NST = B // BB               # 8 stripes
GI = 4                      # input-chunks per activation group
NG = IC // GI               # 2 groups
FD1 = BB * OML              # 2560 free elems per (stripe, chunk)
FDG = GI * FD1              # 10240 free elems per activation tile
NFB = FD1 // 512            # 5 matmul free-blocks per stripe

BF16 = mybir.dt.bfloat16
F32 = mybir.dt.float32


def _build(bias0: float, kv: float, qsv: float, reps: int = 1):
    nc = bacc.Bacc("TRN2", target_bir_lowering=False, debug=False, num_devices=NCORES)

    xT_d = nc.dram_tensor("xT", (P, IC * B), F32, kind="ExternalInput")
    WT_d = nc.dram_tensor("WT", (P, IC * OML), BF16, kind="ExternalInput")
    w2_d = nc.dram_tensor("w2", (P, IC), BF16, kind="ExternalInput")
    out_d = nc.dram_tensor("out", (B, OL), F32, kind="ExternalOutput")

    with tile.TileContext(nc) as tc, ExitStack() as ctx:
        if reps > 1:
            ctx.enter_context(tc.For_i(
                0, reps, 1,
                hint_engines=(mybir.EngineType.DVE, mybir.EngineType.Activation,
                              mybir.EngineType.PE, mybir.EngineType.SP),
            ))
        cpool = ctx.enter_context(tc.tile_pool(name="consts", bufs=1))
        tpool = ctx.enter_context(tc.tile_pool(name="t", bufs=2))
        spool = ctx.enter_context(tc.tile_pool(name="s", bufs=4))
        fpool = ctx.enter_context(tc.tile_pool(name="fin", bufs=1))
        ppool = ctx.enter_context(tc.tile_pool(name="psum", bufs=1, space="PSUM"))

        bias_t = cpool.tile([P, 1], F32)
        nc.gpsimd.memset(bias_t[:], bias0)

        xT = cpool.tile([P, IC * B], F32)
        for icq in range(IC):
            nc.sync.dma_start(xT[:, icq * B:(icq + 1) * B], xT_d[:, icq * B:(icq + 1) * B])
        w2 = cpool.tile([P, IC], BF16)
        nc.sync.dma_start(w2[:], w2_d[:])
        WT = cpool.tile([P, IC * OML], BF16)
        # chunked load so compute can start before the whole weight lands
        for icq in range(IC):
            nc.sync.dma_start(
                WT[:, icq * OML:(icq + 1) * OML], WT_d[:, icq * OML:(icq + 1) * OML]
            )

        d_sb = fpool.tile([B, OML], F32)

        for st in range(NST):
            groups = ([(0, 2), (2, 2), (4, 2), (6, 2)] if st == 0
                      else [(0, GI), (GI, GI)])
            smap = {}
            for ic0, gi in groups:
                t = tpool.tile([P, gi * FD1], BF16)
                for icl in range(gi):
                    ic = ic0 + icl
                    for bl in range(BB):
                        b = st * BB + bl
                        nc.vector.tensor_scalar_mul(
                            t[:, icl * FD1 + bl * OML: icl * FD1 + (bl + 1) * OML],
                            WT[:, ic * OML:(ic + 1) * OML],
                            xT[:, ic * B + b: ic * B + b + 1],
                        )
                s = spool.tile([P, gi * FD1], BF16)
                nc.scalar.activation(
                    s[:], t[:], mybir.ActivationFunctionType.Sigmoid,
                    bias=bias_t[:], scale=0.5,
                )
                for icl in range(gi):
                    smap[ic0 + icl] = (s, icl)

            dps = ppool.tile([1, FD1], F32)
            for ic in range(IC):
                s, icl = smap[ic]
                for fb in range(NFB):
                    nc.tensor.matmul(
                        dps[:, fb * 512:(fb + 1) * 512],
                        w2[:, ic:ic + 1],
                        s[:, icl * FD1 + fb * 512: icl * FD1 + (fb + 1) * 512],
                        start=(ic == 0),
                        stop=(ic == IC - 1),
                    )
            # d for this stripe: psum[0, bl*OML + om] -> d_sb[st*BB+bl, om]
            dcp = tpool.tile([1, FD1], F32, tag="dcp")
            for fb in range(NFB):
                nc.vector.tensor_copy(
                    dcp[:, fb * 512:(fb + 1) * 512], dps[:, fb * 512:(fb + 1) * 512]
                )
            for bl in range(BB):
                nc.sync.dma_start(
                    d_sb[st * BB + bl: st * BB + bl + 1, :],
                    dcp[:, bl * OML:(bl + 1) * OML],
                )

        # membrane: y[b,o] = sum_m sigmoid(d[b,o,m]); out = k*(y - qs)
        sg = fpool.tile([B, OML], F32)
        nc.scalar.activation(sg[:], d_sb[:], mybir.ActivationFunctionType.Sigmoid)
        y = fpool.tile([B, OL], F32)
        nc.vector.reduce_sum(
            y[:], sg[:].rearrange("p (o m) -> p o m", m=M), axis=mybir.AxisListType.X
        )
        outt = fpool.tile([B, OL], F32)
        nc.vector.tensor_scalar(
            outt[:], y[:], kv, -kv * qsv,
            op0=mybir.AluOpType.mult, op1=mybir.AluOpType.add,
        )
        nc.sync.dma_start(out_d[:], outt[:])

    nc.compile()
    return nc


_CACHE: dict = {}


def _get_compiled(bias0: float, kv: float, qsv: float):
    key = (bias0, kv, qsv)
    if key not in _CACHE:
        _CACHE[key] = _build(bias0, kv, qsv)
    return _CACHE[key]


def _prep_inputs(x, Synapse_W, Dendritic_W2):
    xTr = (
        np.ascontiguousarray(x.T)
        .reshape(IC, P, B).transpose(1, 0, 2).reshape(P, IC * B)
        .astype(np.float32)
    )
    w2r = np.ascontiguousarray(Dendritic_W2.reshape(IC, P).T).astype(bfloat16)
    in_maps = []
    for c in range(NCORES):
        Wc = Synapse_W[c * OL:(c + 1) * OL].reshape(OML, IN)
        WTr = (
            np.ascontiguousarray(Wc.T)
            .reshape(IC, P, OML).transpose(1, 0, 2).reshape(P, IC * OML)
            .astype(bfloat16)
        )
        in_maps.append({"xT": xTr, "WT": WTr, "w2": w2r})
    return in_maps


def kernel(x, Synapse_W, Synapse_q, Dendritic_W2, k, qs):
    x = np.asarray(x, dtype=np.float32)
    Synapse_W = np.asarray(Synapse_W, dtype=np.float32)
    Synapse_q = np.asarray(Synapse_q, dtype=np.float32)
    Dendritic_W2 = np.asarray(Dendritic_W2, dtype=np.float32)
    bias0 = -0.5 * float(Synapse_q.reshape(-1)[0])
    kv = float(np.asarray(k).reshape(-1)[0])
    qsv = float(np.asarray(qs).reshape(-1)[0])

    nc = _get_compiled(bias0, kv, qsv)
    in_maps = _prep_inputs(x, Synapse_W, Dendritic_W2)
    res = bass_utils.run_bass_kernel_spmd(nc, in_maps, core_ids=list(range(NCORES)))
    return np.concatenate(
        [res.results[c]["out"] for c in range(NCORES)], axis=1
    ).astype(np.float32)
